# revision 1
# baseline (speedup 1.0000x reference)
"""Trainium2 Bass kernel for nn_AverageItemProfile (scatter_memory).

Strategy: host-side routing groups interactions by item id and buckets items
into "run-length classes" (class r = items with exactly r interactions).
Interactions of each item are laid out contiguously, so on device:
  - the scatter-add of ratings becomes a fixed-stride reduction over r,
  - the running-average update (un-average, add, re-average, clip) is
    item-space elementwise math,
  - the gather back to interactions is a 0-step broadcast access pattern,
  - the weighted sum is an elementwise multiply + innermost reduction.
Everything on device is a regular strided DVE op; the 8 NeuronCores each own
~1/8 of the items of every class (no collectives needed).
"""
import sys

sys.path.insert(0, "/opt/trn_rl_repo")

import numpy as np
from contextlib import ExitStack

import concourse.bass as bass
import concourse.tile as tile
from concourse import bacc, mybir
from concourse.bass_utils import run_bass_kernel_spmd

P = 128            # partitions
NC = 8             # cores
A = 8              # aspects
MIN_R, MAX_R = 1.0, 5.0
TILE_COLS = 512    # target stream columns per on-chip tile
F32 = mybir.dt.float32

_program_cache = {}


def _route(I_ids, n_rows):
    """Host routing: group interactions by item, bucket items by run length.

    Returns the per-interaction and per-item placement plus the class layout.
    Touches only the int index tensor; all float math stays on device.
    """
    B = I_ids.shape[0]
    ids = I_ids.astype(np.int64)
    counts = np.bincount(ids, minlength=n_rows)
    r_int = counts[ids]                                   # run length per interaction
    order = np.argsort((r_int << 21) | ids, kind="stable")
    sid = ids[order]
    new_item = np.empty(B, np.bool_)
    new_item[0] = True
    new_item[1:] = sid[1:] != sid[:-1]
    jglob = np.cumsum(new_item) - 1                       # item rank per interaction
    run_start = np.flatnonzero(new_item)
    l_in_run = np.arange(B) - run_start[jglob]            # slot within the item's run

    item_ids = sid[new_item]                              # distinct ids, (r, id) order
    item_r = counts[item_ids].astype(np.int64)
    rvals, k_list = np.unique(item_r, return_counts=True)

    q_list = -(-k_list // (NC * P))                       # item cols per class
    icol_off = np.concatenate([[0], np.cumsum(q_list)])
    scol_off = np.concatenate([[0], np.cumsum(q_list * rvals)])

    cls_idx = np.searchsorted(rvals, item_r)
    cls_first = np.concatenate([[0], np.cumsum(k_list)])
    jc = np.arange(item_ids.shape[0]) - cls_first[cls_idx]  # rank within class
    core_j = jc % NC
    p_j = (jc // NC) % P
    q_j = jc // (NC * P)
    item_col = icol_off[cls_idx] + q_j
    slot_base = scol_off[cls_idx] + q_j * item_r

    core_t = core_j[jglob]
    p_t = p_j[jglob]
    col_t = slot_base[jglob] + l_in_run

    layout = tuple(zip(rvals.tolist(), q_list.tolist()))
    return dict(
        order=order, item_ids=item_ids,
        core_j=core_j, p_j=p_j, item_col=item_col,
        core_t=core_t, p_t=p_t, col_t=col_t,
        layout=layout,
        tot_icols=int(icol_off[-1]), tot_cols=int(scol_off[-1]),
    )


def _make_chunks(layout):
    """Split class layout into on-chip tiles of <= TILE_COLS stream columns.

    Yields (r, nq, scol0, icol0): nq items/partition starting at those offsets.
    """
    chunks = []
    icol = 0
    scol = 0
    for r, q in layout:
        q_step = max(1, TILE_COLS // r)
        done = 0
        while done < q:
            nq = min(q_step, q - done)
            chunks.append((r, nq, scol, icol))
            icol += nq
            scol += nq * r
            done += nq
    return chunks


def _build_program(layout, tot_icols, tot_cols):
    nc = bacc.Bacc("TRN2", debug=False)
    d_rat = nc.dram_tensor("rat_s", [P, tot_cols * A], F32, kind="ExternalInput")
    d_wts = nc.dram_tensor("wts_s", [P, tot_cols * A], F32, kind="ExternalInput")
    d_par = nc.dram_tensor("par_s", [P, tot_icols * A], F32, kind="ExternalInput")
    d_cnt = nc.dram_tensor("cnt_s", [P, tot_icols], F32, kind="ExternalInput")
    d_out = nc.dram_tensor("preds_s", [P, tot_cols], F32, kind="ExternalOutput")

    with tile.TileContext(nc) as tc, ExitStack() as ctx:
        in_pool = ctx.enter_context(tc.tile_pool(name="in", bufs=3))
        it_pool = ctx.enter_context(tc.tile_pool(name="it", bufs=3))
        tmp_pool = ctx.enter_context(tc.tile_pool(name="tmp", bufs=2))
        out_pool = ctx.enter_context(tc.tile_pool(name="out", bufs=3))

        for r, nq, scol0, icol0 in _make_chunks(layout):
            m = nq * r
            rat = in_pool.tile([P, m * A], F32, tag="rat")
            nc.sync.dma_start(rat[:], d_rat[:, scol0 * A:(scol0 + m) * A])
            wts = in_pool.tile([P, m * A], F32, tag="wts")
            nc.sync.dma_start(wts[:], d_wts[:, scol0 * A:(scol0 + m) * A])
            par = it_pool.tile([P, nq * A], F32, tag="par")
            nc.sync.dma_start(par[:], d_par[:, icol0 * A:(icol0 + nq) * A])
            cnt = it_pool.tile([P, nq], F32, tag="cnt")
            nc.sync.dma_start(cnt[:], d_cnt[:, icol0:icol0 + nq])

            # segment sums over each item's r slots (identity when r == 1)
            if r > 1:
                ssum = tmp_pool.tile([P, nq * A], F32, tag="ssum")
                nc.vector.tensor_reduce(
                    ssum[:].rearrange("p (q a) -> p q a", a=A),
                    rat[:].rearrange("p (q r a) -> p q a r", q=nq, r=r, a=A),
                    axis=mybir.AxisListType.X, op=mybir.AluOpType.add)
            else:
                ssum = rat

            # denom = max(cnt + r, 1); recip = 1/denom
            denom = tmp_pool.tile([P, nq], F32, tag="denom")
            nc.vector.tensor_scalar(denom[:], cnt[:], float(r), 1.0,
                                    op0=mybir.AluOpType.add,
                                    op1=mybir.AluOpType.max)
            recip = tmp_pool.tile([P, nq], F32, tag="recip")
            nc.vector.reciprocal(recip[:], denom[:])

            # work = clip((par*cnt + ssum) * recip, 1, 5)
            work = tmp_pool.tile([P, nq * A], F32, tag="work")
            nc.vector.tensor_tensor(
                work[:].rearrange("p (q a) -> p q a", a=A),
                par[:].rearrange("p (q a) -> p q a", a=A),
                cnt[:].unsqueeze(2).broadcast_to([P, nq, A]),
                op=mybir.AluOpType.mult)
            nc.vector.tensor_tensor(work[:], work[:], ssum[:],
                                    op=mybir.AluOpType.add)
            nc.vector.tensor_tensor(
                work[:].rearrange("p (q a) -> p q a", a=A),
                work[:].rearrange("p (q a) -> p q a", a=A),
                recip[:].unsqueeze(2).broadcast_to([P, nq, A]),
                op=mybir.AluOpType.mult)
            nc.vector.tensor_scalar(work[:], work[:], MIN_R, MAX_R,
                                    op0=mybir.AluOpType.max,
                                    op1=mybir.AluOpType.min)

            # wts *= work (broadcast each item's profile over its r slots)
            nc.vector.tensor_tensor(
                wts[:].rearrange("p (q r a) -> p q r a", q=nq, r=r, a=A),
                wts[:].rearrange("p (q r a) -> p q r a", q=nq, r=r, a=A),
                work[:].rearrange("p (q a) -> p q a", a=A)
                    .unsqueeze(2).broadcast_to([P, nq, r, A]),
                op=mybir.AluOpType.mult)

            pred = out_pool.tile([P, m], F32, tag="pred")
            nc.vector.tensor_reduce(
                pred[:].rearrange("p (q r) -> p q r", r=r),
                wts[:].rearrange("p (q r a) -> p q r a", q=nq, r=r, a=A),
                axis=mybir.AxisListType.X, op=mybir.AluOpType.add)
            nc.sync.dma_start(d_out[:, scol0:scol0 + m], pred[:])

    nc.compile()
    return nc


def _get_program(layout, tot_icols, tot_cols):
    key = (layout, tot_icols, tot_cols)
    if key not in _program_cache:
        _program_cache[key] = _build_program(layout, tot_icols, tot_cols)
    return _program_cache[key]


def _prepare(items_parameters, items_counters, I_ids, A_weights, A_ratings):
    rt = _route(np.asarray(I_ids), items_parameters.shape[0])
    ti, tc_ = rt["tot_icols"], rt["tot_cols"]

    rat_s = np.zeros((NC, P, tc_, A), np.float32)
    wts_s = np.zeros((NC, P, tc_, A), np.float32)
    rat_s[rt["core_t"], rt["p_t"], rt["col_t"]] = np.asarray(A_ratings)[rt["order"]]
    wts_s[rt["core_t"], rt["p_t"], rt["col_t"]] = np.asarray(A_weights)[rt["order"]]
    par_s = np.zeros((NC, P, ti, A), np.float32)
    cnt_s = np.zeros((NC, P, ti), np.float32)
    par_s[rt["core_j"], rt["p_j"], rt["item_col"]] = np.asarray(items_parameters)[rt["item_ids"]]
    cnt_s[rt["core_j"], rt["p_j"], rt["item_col"]] = np.asarray(items_counters)[rt["item_ids"]]

    in_maps = [dict(rat_s=rat_s[c].reshape(P, tc_ * A),
                    wts_s=wts_s[c].reshape(P, tc_ * A),
                    par_s=par_s[c].reshape(P, ti * A),
                    cnt_s=cnt_s[c].reshape(P, ti))
               for c in range(NC)]
    return rt, in_maps


def _run(inputs, trace=False, **kw):
    rt, in_maps = _prepare(**inputs)
    nc = _get_program(rt["layout"], rt["tot_icols"], rt["tot_cols"])
    res = run_bass_kernel_spmd(nc, in_maps, core_ids=list(range(NC)),
                               trace=trace, **kw)
    preds_s = np.stack([res.results[c]["preds_s"] for c in range(NC)])
    B = rt["order"].shape[0]
    out = np.empty(B, np.float32)
    out[rt["order"]] = preds_s[rt["core_t"], rt["p_t"], rt["col_t"]]
    return out, res


def kernel(items_parameters, items_counters, I_ids, A_weights, A_ratings):
    out, _ = _run(dict(items_parameters=items_parameters,
                       items_counters=items_counters,
                       I_ids=I_ids,
                       A_weights=A_weights,
                       A_ratings=A_ratings))
    return out
